# revision 79
# baseline (speedup 1.0000x reference)
"""Trainium2 Bass kernel for nn_Block_with_lora (dense transformer block).

Sharding: 8 cores = 4 batches x 2 token-parity shards (stride-2 over T).
Each core computes its 512 query tokens end-to-end (no collectives);
K/V projections over all 1024 tokens are computed per-core (uniform SPMD
program; all batch/parity dependence lives in the per-core input data).

Layout: all activations transposed [C, T] (host transposes I/O), so every
projection is a natural PE matmul. Attention computes a head PAIR at a time:
the two 64-dim heads of one 128-partition tile issue score matmuls into the
two banks of one [128,1024] PSUM tile from different PE row-groups, so they
run concurrently; one Exp instruction covers both heads' live region.
Softmax denominators ride the AV matmul as an extra ones-column of V.
Causal masking = multiplicative 0/1 band on exp(S) (DVE) for the
diagonal-straddling block; all matmuls only touch the live score region.

All linear-layer biases are folded into the LoRA rank: z gets a 17th row of
ones and B gets a 17th row holding the bias, so PSUM drains are plain DVE
copies/adds and no ones-row bias matmuls exist.

Small host tensors (LN affine columns, per-column biases, LoRA A, selector
matrices) are pre-arranged on the host into partition-major layouts so every
DMA is a handful of fat descriptors; they ride the ACT HWDGE queue, leaving
the sync queue for x/weights.
"""

import sys

sys.path.insert(0, "/opt/trn_rl_repo")

import numpy as np
import ml_dtypes
from contextlib import ExitStack

BF = ml_dtypes.bfloat16

C = 1024
H = 16
DH = 64
R = 16
RA = R + 1  # lora rank + ones row (bias folding)
SCALE = 1.0 / R
T = 1024
TQ = 512
NT = 8  # C / 128
EPS = 1e-5
NCORES = 8

_PROG = None

# bcols column layout: g1[0:8] b1[8:16] g2[16:24] b2[24:32] bpr[32:40] bfc[40:72]
BC_G1, BC_B1, BC_G2, BC_B2, BC_BPR, BC_BFC = 0, 8, 16, 24, 32, 40
# apack tag order
A_TAGS = ["a_sa", "a_sp", "a_cq", "a_ck", "a_cp"]


def _build_program():
    import concourse.bass as bass
    import concourse.tile as tile
    from concourse import mybir, bacc

    f32 = mybir.dt.float32
    bf16 = mybir.dt.bfloat16
    AF = mybir.ActivationFunctionType
    AL = mybir.AluOpType

    nc = bacc.Bacc("TRN2", target_bir_lowering=False, debug=False)

    def din(name, shape, dt=f32):
        return nc.dram_tensor(name, shape, dt, kind="ExternalInput").ap()

    xT_d = din("xT", [C, T])
    xqT_d = din("xqT", [C, TQ])
    fT_d = din("fT", [C, T])
    band_d = din("band2", [128, 128], bf16)  # 0/1 multiplicative, duplicated 2x

    w_d = {}
    for n in ["wq", "wk", "wv", "wsp", "wcq", "wck", "wcv", "wcp"]:
        w_d[n] = din(n, [C, C], bf16)
    w_d["wfc"] = din("wfc", [C, 4 * C], bf16)
    w_d["wpr"] = din("wpr", [4 * C, C], bf16)
    apack_d = din("apack", [128, len(A_TAGS) * NT * R], bf16)
    b_d = {
        n: din(n, [RA, C], bf16)
        for n in ["b_saq", "b_sak", "b_sav", "b_sp", "b_cq", "b_ckk", "b_ckv", "b_cp"]
    }
    bcols_d = din("bcols", [128, 72], f32)
    # denominator-selector rows: pair mi uses rows 2*(mi%4), 2*(mi%4)+1 of an
    # 8-row block; pairs 0-3 and 4-7 land in separate base-0 tiles so each
    # half-tail's recip + selector matmul run at base partition 0
    sel_d = din("sel", [8, NT * 128], f32)

    outT_d = nc.dram_tensor("outT", [C, TQ], f32, kind="ExternalOutput").ap()

    with tile.TileContext(nc) as tc, ExitStack() as ctx:

        def pool(name, bufs, space=None):
            kw = dict(name=name, bufs=bufs)
            if space:
                kw["space"] = space
            return ctx.enter_context(tc.tile_pool(**kw))

        # SBUF pools
        big32 = pool("big32", 2)        # [128,1024] f32: LN temps
        xbpool = pool("xbpool", 8)      # [128,1024] bf16: x in bf16 (persist ph1)
        acts = pool("acts", 8)          # [128,1024] bf16: lnb then fb
        lnsm = pool("lnsm", 8)          # [128,512] bf16: lnown -> ln1b -> ln2
        qpool = pool("qpool", 8)        # [128,512] bf16: qT -> q2T
        kpool = pool("kpool", 8)        # [128,1024] bf16: kT -> mlp hidden 0:16
        k2pool = pool("k2pool", 8)      # [128,1024] bf16: k2T -> mlp hidden 16:32
        vpool = pool("vpool", 8)        # [128,1040] bf16: V -> V2
        opool = pool("opool", 8)        # [128,512] bf16: oT -> o2T
        rpool = pool("rpool", 8)        # [128,512] f32: residual (persist)
        wpool = pool("wpool", 9)        # [128,512] bf16: weight chunks
        wpool2 = pool("wpool2", 8)      # [128,512] bf16: woven k2/cv chunks
        epool = pool("epool", 2)        # [128,1024] bf16: exp(S) head pair
        sqpool = pool("sqpool", 2)      # squares for LN var
        sbig = pool("sbig", 2)          # [128,1024] f32: LN mean/rstd bcast
        rows = pool("rows", 2)          # [1,1024] f32: LN stat rows
        rrows = pool("rrows", 2)        # [1,512] f32: softmax denom rows
        recb = pool("recb", 2)          # [128,512] f32: recip bcast
        dallp = pool("dallp", 2)        # [16,512] f32: batched softmax denoms
        outfp = pool("outfp", 2)        # [128,512] f32: final out staging
        zpool = pool("zpool", 1)        # [17,*] bf16: lora z (1 slot per tag)
        lorab = pool("lorab", 4)        # [17,1024] bf16: lora B rows (aug)
        loraa = pool("loraa", 1)        # [128,640] bf16: all lora A chunks
        smalls = pool("smalls", 1)      # small constants per tag
        onesp = pool("onesp", 1)

        # PSUM pools: 4 + 2 + 2 = 8 banks
        ps = pool("ps", 2, space="PSUM")   # [128,1024] f32: S pair tiles, LN stats, pr acc
        po = pool("po", 2, space="PSUM")   # [65..128,512] f32: attn out acc, fc
        pp = pool("pp", 2, space="PSUM")   # [128,512] f32: projections, z, fc

        # ---- constants (tiny; ACT HWDGE queue for all small loads) ----
        ones_c16 = onesp.tile([128, 1], bf16, tag="oc16")
        nc.gpsimd.memset(ones_c16[:], 1.0)
        ones_r32 = onesp.tile([1, 128], f32, tag="or32")
        nc.gpsimd.memset(ones_r32[:], 1.0)
        eps_t = onesp.tile([1, 1], f32, tag="eps")
        nc.gpsimd.memset(eps_t[:], EPS)
        warm_t = onesp.tile([128, 512], bf16, tag="warm")
        nc.gpsimd.memset(warm_t[:], 0.125)

        bcols_t = smalls.tile([128, 72], f32, tag="bcols")
        nc.scalar.dma_start(bcols_t[:], bcols_d[:, :])
        band_t = smalls.tile([128, 128], bf16, tag="band")
        nc.scalar.dma_start(band_t[:], band_d[:, :])
        sel_t = smalls.tile([8, NT * 128], f32, tag="sel")
        nc.scalar.dma_start(sel_t[:], sel_d[:, :])

        apack_t = loraa.tile([128, len(A_TAGS) * NT * R], bf16, tag="loraa")
        nc.scalar.dma_start(apack_t[:], apack_d[:, :])

        def bcol(base, k):
            return bcols_t[:, base + k:base + k + 1]

        # ---- PE warm-up burst: keep HAM busy during initial DMA ----
        for _ in range(28):
            wp = pp.tile([128, 512], f32, tag="pp")
            nc.tensor.matmul(wp[:], warm_t[:, 0:128], warm_t[:], start=True, stop=True)

        dma_rr = [0]

        def wdma(dst, src):
            # spread weight streaming across two DMA queues
            eng = (nc.sync, nc.gpsimd)[dma_rr[0] % 2]
            dma_rr[0] += 1
            eng.dma_start(dst, src)

        def load_lora_b(name):
            t = lorab.tile([RA, C], bf16, tag="lorab")
            nc.scalar.dma_start(t[:], b_d[name][:, :])
            return t

        def a_sl(tag_idx, k):
            off = (tag_idx * NT + k) * R
            return apack_t[:, off:off + R]

        # =============== helpers ===============
        def bcast_row(row, out_sb, Tn):
            # broadcast [1, Tn] f32 row to [128, Tn] SBUF via K=1 PE matmul
            for h in range(Tn // 512):
                sl = slice(h * 512, (h + 1) * 512)
                bp = pp.tile([128, 512], f32, tag="pp")
                nc.tensor.matmul(bp[:], ones_r32[:], row[0:1, sl], start=True, stop=True)
                nc.vector.tensor_copy(out_sb[:, sl], bp[:])

        def ln_stats_and_norm(src_tiles, gbase, bbase, out_tiles):
            """LayerNorm over channel (partition) dim; src 8x[128,512] f32 persistent."""
            mean_ps = ps.tile([1, TQ], f32, tag="ps")
            sq_ps = ps.tile([1, TQ], f32, tag="ps")
            for k in range(NT):
                xb = sqpool.tile([128, TQ], bf16, tag="sqo")
                nc.vector.tensor_copy(xb[:], src_tiles[k][:])
                sq = sqpool.tile([128, TQ], bf16, tag="sqo")
                nc.vector.tensor_mul(sq[:], xb[:], xb[:])
                nc.tensor.matmul(mean_ps[:], ones_c16[:], xb[:],
                                 start=(k == 0), stop=(k == NT - 1))
                nc.tensor.matmul(sq_ps[:], ones_c16[:], sq[:],
                                 start=(k == 0), stop=(k == NT - 1))
            mean_row = rows.tile([1, TQ], f32, tag="rows")
            rstd_row = rows.tile([1, TQ], f32, tag="rows")
            nc.vector.tensor_scalar_mul(mean_row[:], mean_ps[:], 1.0 / C)
            nc.vector.tensor_mul(rstd_row[:], mean_row[:], mean_row[:])
            nc.vector.scalar_tensor_tensor(rstd_row[:], sq_ps[:], 1.0 / C, rstd_row[:],
                                           op0=AL.mult, op1=AL.subtract)
            nc.scalar.activation(rstd_row[:], rstd_row[:], AF.Sqrt, bias=eps_t[:])
            nc.vector.reciprocal_approx_fast(rstd_row[:], rstd_row[:])
            mb = sbig.tile([128, TQ], f32, tag="sbig")
            rb = sbig.tile([128, TQ], f32, tag="sbig")
            bcast_row(mean_row, mb, TQ)
            bcast_row(rstd_row, rb, TQ)
            for k in range(NT):
                t1 = big32.tile([128, TQ], f32, tag="big32")
                nc.vector.tensor_sub(t1[:], src_tiles[k][:], mb[:])
                nc.vector.tensor_mul(t1[:], t1[:], rb[:])
                nc.scalar.activation(out_tiles[k][:], t1[:], AF.Identity,
                                     bias=bcol(bbase, k), scale=bcol(gbase, k))

        def compute_z(atag, rhs_tiles, Tn, tag):
            """z^T = A-proj of activations + ones row: [17, Tn] bf16."""
            z_sb = zpool.tile([RA, Tn], bf16, tag=tag)
            # row R stays 1.0 (bias row); rows 0:R are overwritten below
            nc.gpsimd.memset(z_sb[:], 1.0)
            for h in range(Tn // 512):
                sl = slice(h * 512, (h + 1) * 512)
                zp = pp.tile([R, 512], f32, tag="pp")
                for k in range(NT):
                    nc.tensor.matmul(zp[:], a_sl(atag, k), rhs_tiles[k][:, sl],
                                     start=(k == 0), stop=(k == NT - 1))
                nc.vector.tensor_copy(z_sb[0:R, sl], zp[:])
            return z_sb

        def projT(wname, rhs_tiles, Tn, z_sb, bname, out_cb, pools=None, weng=None):
            """out^T tiles via PE; lora(+bias) + callback per (M-tile, t-half)."""
            if pools is None:
                pools = ((pp, "pp"),)
            b_t = load_lora_b(bname)
            pcnt = 0
            for mh in range(2):  # c_out halves of 512
                wts = []
                for k in range(NT):
                    wt = wpool.tile([128, 512], bf16, tag="wpool")
                    src = w_d[wname][k * 128:(k + 1) * 128, mh * 512:(mh + 1) * 512]
                    if weng is not None:
                        weng.dma_start(wt[:], src)
                    else:
                        wdma(wt[:], src)
                    wts.append(wt)
                for ml in range(4):
                    mi = mh * 4 + ml
                    for h in range(Tn // 512):
                        sl = slice(h * 512, (h + 1) * 512)
                        pl, ptag = pools[pcnt % len(pools)]
                        pcnt += 1
                        pt = pl.tile([128, 512], f32, tag=ptag)
                        for k in range(NT):
                            nc.tensor.matmul(pt[:], wts[k][:, ml * 128:(ml + 1) * 128],
                                             rhs_tiles[k][:, sl], start=(k == 0), stop=False)
                        nc.tensor.matmul(pt[:], b_t[:, mi * 128:(mi + 1) * 128],
                                         z_sb[:, sl], start=False, stop=True)
                        out_cb(mi, pt, h)

        def proj_V(wname, lhs_tiles, z_sb, bname, v_tiles, pools=None):
            """V natural [t, d] with activations stationary; lora row16 adds bias."""
            if pools is None:
                pools = ((pp, "pp"),)
            b_t = load_lora_b(bname)
            pcnt = 0
            for dh in range(2):
                sl = slice(dh * 512, (dh + 1) * 512)
                wts = []
                for k in range(NT):
                    wt = wpool.tile([128, 512], bf16, tag="wpool")
                    wdma(wt[:], w_d[wname][k * 128:(k + 1) * 128, sl])
                    wts.append(wt)
                for tt in range(NT):
                    pl, ptag = pools[pcnt % len(pools)]
                    pcnt += 1
                    pt = pl.tile([128, 512], f32, tag=ptag)
                    for k in range(NT):
                        nc.tensor.matmul(pt[:], lhs_tiles[k][:, tt * 128:(tt + 1) * 128],
                                         wts[k][:], start=(k == 0), stop=False)
                    nc.tensor.matmul(pt[:], z_sb[:, tt * 128:(tt + 1) * 128],
                                     b_t[:, sl], start=False, stop=True)
                    dest = v_tiles[tt][:, dh * 520:(dh + 1) * 520]
                    dest = dest.rearrange("p (h d) -> p h d", d=65)[:, :, 0:64]
                    nc.vector.tensor_copy(dest, pt[:])

        def attention_pair(mi, q_tiles, k_tiles, v_tiles, o_tiles, dallA, dallB):
            # Head pair per M-tile: scores for heads (2mi, 2mi+1) go to the two
            # banks of one [128,1024] PSUM tile from different PE row groups
            # (concurrent); one Exp covers both; AV accumulates each head with
            # a ones-column for the softmax denominator.
            hA, hB = 2 * mi, 2 * mi + 1
            opA = po.tile([65, 512], f32, tag="po")
            opB = po.tile([65, 512], f32, tag="po")
            sts = {}

            def emit_S(kj):
                q0 = 64 * kj
                stP = ps.tile([128, 1024], f32, tag="ps")
                nc.tensor.matmul(
                    stP[:, q0:512],
                    k_tiles[mi][0:64, kj * 128:(kj + 1) * 128],
                    q_tiles[mi][0:64, q0:512], start=True, stop=True)
                nc.tensor.matmul(
                    stP[:, 512 + q0:1024],
                    k_tiles[mi][64:128, kj * 128:(kj + 1) * 128],
                    q_tiles[mi][64:128, q0:512], start=True, stop=True)
                sts[kj] = stP

            emit_S(0)
            for kj in range(NT):
                if kj + 1 < NT:
                    emit_S(kj + 1)  # S one step ahead: PE busy during Exp(kj)
                q0 = 64 * kj
                stP = sts.pop(kj)
                et = epool.tile([128, 1024], bf16, tag="epool")
                st3 = stP[:].rearrange("p (two c) -> p two c", two=2)[:, :, q0:512]
                et3 = et[:].rearrange("p (two c) -> p two c", two=2)[:, :, q0:512]
                nc.scalar.activation(et3, st3, AF.Exp)
                # multiplicative causal band on the diagonal-straddling queries
                eb = et[:].rearrange("p (two c) -> p two c", two=2)[:, :, q0:q0 + 64]
                bb = band_t[:].rearrange("p (two c) -> p two c", two=2)
                nc.vector.tensor_mul(eb, eb, bb)
                nc.tensor.matmul(
                    opA[:] if kj == 0 else opA[:, q0:512],
                    v_tiles[kj][:, 65 * hA:65 * hA + 65],
                    et[:, q0:512], start=(kj == 0), stop=(kj == NT - 1))
                nc.tensor.matmul(
                    opB[:] if kj == 0 else opB[:, q0:512],
                    v_tiles[kj][:, 65 * hB:65 * hB + 65],
                    et[:, 512 + q0:1024], start=(kj == 0), stop=(kj == NT - 1))
            # stash raw (unnormalized) head outputs + denominator rows
            nc.vector.tensor_copy(o_tiles[mi][0:64, :], opA[0:64, :])
            nc.vector.tensor_copy(o_tiles[mi][64:128, :], opB[0:64, :])
            dall = dallA if mi < 4 else dallB
            rA = 2 * (mi % 4)
            rrA = rrows.tile([1, 512], f32, tag="rrows")
            nc.vector.tensor_copy(rrA[:], opA[64:65, :])
            nc.sync.dma_start(dall[rA:rA + 1, :], rrA[:])
            rrB = rrows.tile([1, 512], f32, tag="rrows")
            nc.vector.tensor_copy(rrB[:], opB[64:65, :])
            nc.sync.dma_start(dall[rA + 1:rA + 2, :], rrB[:])

        def tail_first_half(dallA, o_tiles):
            # rescale pairs 0-3 while pairs 4-7 still run
            nc.vector.reciprocal_approx_fast(dallA[:], dallA[:])
            for mi2 in range(4):
                bp = pp.tile([128, 512], f32, tag="pp")
                nc.tensor.matmul(bp[:], sel_t[:, mi2 * 128:(mi2 + 1) * 128],
                                 dallA[:], start=True, stop=True)
                rbc = recb.tile([128, 512], f32, tag="recb")
                nc.vector.tensor_copy(rbc[:], bp[:])
                nc.vector.tensor_mul(o_tiles[mi2][:], o_tiles[mi2][:], rbc[:])

        def attention_tail(dallB, o_tiles, atag, ztag):
            # second half of the rescale + the follow-on lora-z matmuls
            nc.vector.reciprocal_approx_fast(dallB[:], dallB[:])
            z_sb = zpool.tile([RA, TQ], bf16, tag=ztag)
            nc.gpsimd.memset(z_sb[:], 1.0)
            zp = po.tile([R, 512], f32, tag="po")
            for mi2 in range(4, NT):
                bp = pp.tile([128, 512], f32, tag="pp")
                nc.tensor.matmul(bp[:], sel_t[:, mi2 * 128:(mi2 + 1) * 128],
                                 dallB[:], start=True, stop=True)
                rbc = recb.tile([128, 512], f32, tag="recb")
                nc.vector.tensor_copy(rbc[:], bp[:])
                nc.vector.tensor_mul(o_tiles[mi2][:], o_tiles[mi2][:], rbc[:])
            for k in range(NT):
                nc.tensor.matmul(zp[:], a_sl(atag, k), o_tiles[k][:],
                                 start=(k == 0), stop=(k == NT - 1))
            nc.vector.tensor_copy(z_sb[0:R, :], zp[:])
            return z_sb

        def fused_attention(wname, rhs_tiles, z_sb, bname, q_out, k_tiles,
                            v_tiles, o_tiles, thunks, atag, ztag, warm_fill=0):
            """Per M-tile: optionally project q (8 w-MMs + lora), then run the
            attention head pair; weave independent work (thunks) between pairs
            so the PE stays dense while the softmax chain runs. Returns the
            lora z of the attention output (computed inside the tail)."""
            dallA = dallp.tile([8, 512], f32, tag="dallp")
            dallB = dallp.tile([8, 512], f32, tag="dallp")
            tq = list(thunks)
            ti = [0]

            def pop(n):
                for _ in range(n):
                    if ti[0] < len(tq):
                        tq[ti[0]]()
                        ti[0] += 1

            if wname is None:
                for mi in range(NT):
                    attention_pair(mi, q_out, k_tiles, v_tiles, o_tiles, dallA, dallB)
                    pop(5)
                    if mi == 4:
                        tail_first_half(dallA, o_tiles)
            else:
                b_t = load_lora_b(bname)
                for mh in range(2):
                    wts = []
                    for k in range(NT):
                        wt = wpool.tile([128, 512], bf16, tag="wpool")
                        wdma(wt[:], w_d[wname][k * 128:(k + 1) * 128,
                                               mh * 512:(mh + 1) * 512])
                        wts.append(wt)
                    for ml in range(4):
                        mi = mh * 4 + ml
                        pt = pp.tile([128, 512], f32, tag="pp")
                        for k in range(NT):
                            nc.tensor.matmul(pt[:], wts[k][:, ml * 128:(ml + 1) * 128],
                                             rhs_tiles[k][:], start=(k == 0), stop=False)
                        nc.tensor.matmul(pt[:], b_t[:, mi * 128:(mi + 1) * 128],
                                         z_sb[:], start=False, stop=True)
                        nc.vector.tensor_copy(q_out[mi][:], pt[:])
                        attention_pair(mi, q_out, k_tiles, v_tiles, o_tiles, dallA, dallB)
                        pop(5)
                        # dense PE filler pinned after this pair's output: a
                        # sustained-busy stretch lets HAM restore full clock
                        for _ in range(warm_fill):
                            wp = pp.tile([128, 512], f32, tag="pp")
                            nc.tensor.matmul(wp[:], warm_t[:, 0:128],
                                             o_tiles[mi][:], start=True, stop=True)
                        if mi == 4:
                            tail_first_half(dallA, o_tiles)
            pop(len(tq))
            return attention_tail(dallB, o_tiles, atag, ztag)

        # ===== phase 0: residual + LN(own) first (enables early q projection) =====
        resid = []
        for k in range(NT):
            rt = rpool.tile([128, TQ], f32, tag="rpool")
            nc.sync.dma_start(rt[:], xqT_d[k * 128:(k + 1) * 128, :])
            resid.append(rt)
        lnown = [lnsm.tile([128, TQ], bf16, tag="lnsm", name=f"lnown{i}") for i in range(NT)]
        ln_stats_and_norm(resid, BC_G1, BC_B1, lnown)
        z_own = compute_z(0, lnown, TQ, "zsm")

        # ===== phase 1: LN1 over full x (x cast to bf16 during DMA) =====
        mean_ps = ps.tile([1, T], f32, tag="ps")
        sq_ps = ps.tile([1, T], f32, tag="ps")
        xbt = []
        for k in range(NT):
            xb = xbpool.tile([128, 1040], bf16, tag="xb")
            if k % 2 == 0:
                nc.gpsimd.dma_start(xb[:, 0:T], xT_d[k * 128:(k + 1) * 128, :])
            else:
                # split x across both DMA queues: stage f32 on sync, cast on DVE
                xt = big32.tile([128, T], f32, tag="big32")
                nc.sync.dma_start(xt[:], xT_d[k * 128:(k + 1) * 128, :])
                nc.vector.tensor_copy(xb[:, 0:T], xt[:])
            xbt.append(xb)
            sq = sqpool.tile([128, T], bf16, tag="sqf")
            nc.vector.tensor_mul(sq[:], xb[:, 0:T], xb[:, 0:T])
            for hh in range(2):
                sl = slice(hh * 512, (hh + 1) * 512)
                nc.tensor.matmul(mean_ps[0:1, sl], ones_c16[:], xb[:, 0:T][:, sl],
                                 start=(k == 0), stop=(k == NT - 1))
                nc.tensor.matmul(sq_ps[0:1, sl], ones_c16[:], sq[:, sl],
                                 start=(k == 0), stop=(k == NT - 1))
        mean_row = rows.tile([1, T], f32, tag="rows")
        rstd_row = rows.tile([1, T], f32, tag="rows")
        nc.vector.tensor_scalar_mul(mean_row[:], mean_ps[:], 1.0 / C)
        nc.vector.tensor_mul(rstd_row[:], mean_row[:], mean_row[:])
        nc.vector.scalar_tensor_tensor(rstd_row[:], sq_ps[:], 1.0 / C, rstd_row[:],
                                       op0=AL.mult, op1=AL.subtract)
        nc.scalar.activation(rstd_row[:], rstd_row[:], AF.Sqrt, bias=eps_t[:])
        nc.vector.reciprocal_approx_fast(rstd_row[:], rstd_row[:])
        mb_f = sbig.tile([128, T], f32, tag="sbig")
        rb_f = sbig.tile([128, T], f32, tag="sbig")
        bcast_row(mean_row, mb_f, T)
        bcast_row(rstd_row, rb_f, T)
        lnb = [acts.tile([128, T], bf16, tag="acts", name=f"lnb{i}") for i in range(NT)]
        for k in range(NT):
            t1 = big32.tile([128, T], f32, tag="big32")
            nc.vector.tensor_sub(t1[:], xbt[k][:, 0:T], mb_f[:])
            nc.vector.tensor_mul(t1[:], t1[:], rb_f[:])
            nc.scalar.activation(lnb[k][:], t1[:], AF.Identity,
                                 bias=bcol(BC_B1, k), scale=bcol(BC_G1, k))

        # ===== phase 1b: self q projection (overlaps the LN normalize chain) =====
        qT = [qpool.tile([128, TQ], bf16, tag="qpool", name=f"qT{i}") for i in range(NT)]

        def q_cb(mi, pt, h):
            nc.vector.tensor_copy(qT[mi][:], pt[:])

        projT("wq", lnown, TQ, z_own, "b_saq", q_cb,
              pools=((pp, "pp"), (po, "po"), (ps, "ps")), weng=nc.sync)

        # ===== phase 2: self K/V (dense PE block while feature streams in) =====
        z_sa = compute_z(0, lnb, T, "zbig")

        kT = [kpool.tile([128, T], bf16, tag="kpool", name=f"kT{i}") for i in range(NT)]

        def k_cb(mi, pt, h):
            nc.vector.tensor_copy(kT[mi][:, h * 512:(h + 1) * 512], pt[:])

        projT("wk", lnb, T, z_sa, "b_sak", k_cb,
              pools=((pp, "pp"), (po, "po"), (ps, "ps")))

        vt = [vpool.tile([128, 1040], bf16, tag="vpool", name=f"vt{i}") for i in range(NT)]
        for tt in range(NT):
            nc.gpsimd.memset(vt[tt][:, 64:1040:65], 1.0)
        proj_V("wv", lnb, z_sa, "b_sav", vt, pools=((pp, "pp"), (po, "po"), (ps, "ps")))

        # ===== phase 3: cross K/V built as thunks, woven into self-attention =====
        fb = [acts.tile([128, T], bf16, tag="acts", name=f"fb{i}") for i in range(NT)]
        for k in range(NT):
            nc.gpsimd.dma_start(fb[k][:], fT_d[k * 128:(k + 1) * 128, :])
        z_ck = compute_z(3, fb, T, "zbig2")
        k2T = [k2pool.tile([128, T], bf16, tag="k2pool", name=f"k2T{i}") for i in range(NT)]
        # cross V reuses the dead x (bf16) buffers
        v2t = [xbpool.tile([128, 1040], bf16, tag="xb", name=f"v2t{i}") for i in range(NT)]
        for tt in range(NT):
            nc.gpsimd.memset(v2t[tt][:, 64:1040:65], 1.0)

        thunks = []
        b_k2 = load_lora_b("b_ckk")
        b_cv = load_lora_b("b_ckv")

        def mk_k2(mh):
            wts = []

            def load():
                for k in range(NT):
                    wt = wpool2.tile([128, 512], bf16, tag="wpool2")
                    wdma(wt[:], w_d["wck"][k * 128:(k + 1) * 128,
                                           mh * 512:(mh + 1) * 512])
                    wts.append(wt)
            thunks.append(load)
            for ml in range(4):
                for h in range(2):
                    def grp(ml=ml, h=h):
                        mi = mh * 4 + ml
                        sl = slice(h * 512, (h + 1) * 512)
                        pt = pp.tile([128, 512], f32, tag="pp")
                        for k in range(NT):
                            nc.tensor.matmul(pt[:], wts[k][:, ml * 128:(ml + 1) * 128],
                                             fb[k][:, sl], start=(k == 0), stop=False)
                        nc.tensor.matmul(pt[:], b_k2[:, mi * 128:(mi + 1) * 128],
                                         z_ck[:, sl], start=False, stop=True)
                        nc.vector.tensor_copy(k2T[mi][:, sl], pt[:])
                    thunks.append(grp)

        def mk_cv(dh):
            wts = []

            def load():
                for k in range(NT):
                    wt = wpool2.tile([128, 512], bf16, tag="wpool2")
                    wdma(wt[:], w_d["wcv"][k * 128:(k + 1) * 128,
                                           dh * 512:(dh + 1) * 512])
                    wts.append(wt)
            thunks.append(load)
            for tt in range(NT):
                def grp(tt=tt):
                    pt = pp.tile([128, 512], f32, tag="pp")
                    for k in range(NT):
                        nc.tensor.matmul(pt[:], fb[k][:, tt * 128:(tt + 1) * 128],
                                         wts[k][:], start=(k == 0), stop=False)
                    nc.tensor.matmul(pt[:], z_ck[:, tt * 128:(tt + 1) * 128],
                                     b_cv[:, dh * 512:(dh + 1) * 512],
                                     start=False, stop=True)
                    dest = v2t[tt][:, dh * 520:(dh + 1) * 520]
                    dest = dest.rearrange("p (h d) -> p h d", d=65)[:, :, 0:64]
                    nc.vector.tensor_copy(dest, pt[:])
                thunks.append(grp)

        mk_k2(0)
        mk_cv(0)
        mk_k2(1)
        mk_cv(1)

        # ===== phase 4: self attention + woven cross K/V thunks =====
        oT = [opool.tile([128, TQ], bf16, tag="opool", name=f"oT{i}") for i in range(NT)]
        z_sp = fused_attention(None, None, None, None, qT, kT, vt, oT, thunks,
                               1, "zsm")

        # ===== phase 6: self proj + residual =====
        def sp_cb(mi, pt, h):
            nc.vector.tensor_add(resid[mi][:], pt[:], resid[mi][:])

        projT("wsp", oT, TQ, z_sp, "b_sp", sp_cb,
              pools=((pp, "pp"), (po, "po"), (ps, "ps")))

        # ===== phase 7: LN1 on updated own tokens =====
        ln1b = [lnsm.tile([128, TQ], bf16, tag="lnsm", name=f"ln1b{i}") for i in range(NT)]
        ln_stats_and_norm(resid, BC_G1, BC_B1, ln1b)

        # ===== phase 8+9: fused cross q-projection + cross attention =====
        z_cq = compute_z(2, ln1b, TQ, "zsm")
        q2T = [qpool.tile([128, TQ], bf16, tag="qpool", name=f"q2T{i}") for i in range(NT)]
        o2T = [opool.tile([128, TQ], bf16, tag="opool", name=f"o2T{i}") for i in range(NT)]
        z_cp = fused_attention("wcq", ln1b, z_cq, "b_cq", q2T, k2T, v2t, o2T, [],
                               4, "zsm")

        # =============== phase 10: cross proj + residual ===============
        def cp_cb(mi, pt, h):
            nc.vector.tensor_add(resid[mi][:], pt[:], resid[mi][:])

        projT("wcp", o2T, TQ, z_cp, "b_cp", cp_cb,
              pools=((pp, "pp"), (po, "po"), (ps, "ps")))

        # =============== phase 11: LN2 + MLP (single pass, N=512) ===============
        ln2 = [lnsm.tile([128, TQ], bf16, tag="lnsm", name=f"ln2_{i}") for i in range(NT)]
        ln_stats_and_norm(resid, BC_G2, BC_B2, ln2)

        # fc: hidden chunks land in the dead kT/k2T buffers (32 x [128,512])
        mtiles = (
            [kpool.tile([128, T], bf16, tag="kpool", name=f"mk{i}") for i in range(NT)]
            + [k2pool.tile([128, T], bf16, tag="k2pool", name=f"mk2{i}") for i in range(NT)]
        )

        def m_sl(mi):
            t = mtiles[mi // 2]
            return t[:, (mi % 2) * 512:(mi % 2 + 1) * 512]

        fcp = ((pp, "pp"), (po, "po"))
        pcnt = 0
        for grp in range(8):
            wts = []
            for k in range(NT):
                # fc chunks ride the wpool2 ring (idle since the self-attn
                # thunks) so they prefetch during cross attention
                wt = wpool2.tile([128, 512], bf16, tag="wpool2")
                wdma(wt[:], w_d["wfc"][k * 128:(k + 1) * 128,
                                       grp * 512:(grp + 1) * 512])
                wts.append(wt)
            for ml in range(4):
                mi = grp * 4 + ml
                pl, ptag = fcp[pcnt % 2]
                pcnt += 1
                pt = pl.tile([128, 512], f32, tag=ptag)
                for k in range(NT):
                    nc.tensor.matmul(pt[:], wts[k][:, ml * 128:(ml + 1) * 128],
                                     ln2[k][:], start=(k == 0), stop=(k == NT - 1))
                nc.scalar.activation(m_sl(mi), pt[:], AF.Gelu_apprx_tanh,
                                     bias=bcol(BC_BFC, mi))

        for quad in range(2):
            qts = [ps.tile([128, 1024], f32, tag="ps", name=f"prq{quad}_{j}")
                   for j in range(2)]
            for k in range(32):
                wt = wpool.tile([128, 512], bf16, tag="wpool")
                wdma(wt[:], w_d["wpr"][k * 128:(k + 1) * 128,
                                       quad * 512:(quad + 1) * 512])
                for j in range(4):
                    nc.tensor.matmul(qts[j // 2][:, (j % 2) * 512:(j % 2 + 1) * 512],
                                     wt[:, j * 128:(j + 1) * 128],
                                     m_sl(k), start=(k == 0), stop=(k == 31))
            for j in range(4):
                mi = quad * 4 + j
                of = outfp.tile([128, TQ], f32, tag="outfp")
                nc.vector.scalar_tensor_tensor(of[:],
                                               qts[j // 2][:, (j % 2) * 512:(j % 2 + 1) * 512],
                                               bcol(BC_BPR, mi),
                                               resid[mi][:],
                                               op0=AL.add, op1=AL.add)
                nc.sync.dma_start(outT_d[mi * 128:(mi + 1) * 128, :], of[:])

    nc.compile()
    return nc


def _get_program():
    global _PROG
    if _PROG is None:
        _PROG = _build_program()
    return _PROG


def _prep_shared(inputs):
    g = {}

    def bf(a):
        return np.ascontiguousarray(np.asarray(a, dtype=np.float32)).astype(BF)

    def f(a):
        return np.ascontiguousarray(np.asarray(a, dtype=np.float32))

    qw, kw, vw = (inputs["sa_qkv_w"][i * C:(i + 1) * C] for i in range(3))
    qb, kb, vb = (inputs["sa_qkv_b"][i * C:(i + 1) * C] for i in range(3))
    qlb, klb, vlb = (inputs["sa_qkv_lb"][i * C:(i + 1) * C] for i in range(3))
    inv = 1.0 / np.sqrt(DH)

    def baug(lb_T_scaled, bias):
        # [R+1, C]: lora B rows + bias row (pairs with the ones row in z)
        return bf(np.concatenate([np.asarray(lb_T_scaled),
                                  np.asarray(bias).reshape(1, C)], axis=0))

    g["wq"] = bf(np.asarray(qw).T * inv)
    g["wk"] = bf(np.asarray(kw).T)
    g["wv"] = bf(np.asarray(vw).T)
    g["b_saq"] = baug(np.asarray(qlb).T * (SCALE * inv), np.asarray(qb) * inv)
    g["b_sak"] = baug(np.asarray(klb).T * SCALE, kb)
    g["b_sav"] = baug(np.asarray(vlb).T * SCALE, vb)

    g["wsp"] = bf(np.asarray(inputs["sa_proj_w"]).T)
    g["b_sp"] = baug(np.asarray(inputs["sa_proj_lb"]).T * SCALE, inputs["sa_proj_b"])

    g["wcq"] = bf(np.asarray(inputs["ca_q_w"]).T * inv)
    g["b_cq"] = baug(np.asarray(inputs["ca_q_lb"]).T * (SCALE * inv),
                     np.asarray(inputs["ca_q_b"]) * inv)

    ckw, cvw = inputs["ca_kv_w"][0:C], inputs["ca_kv_w"][C:2 * C]
    ckb, cvb = inputs["ca_kv_b"][0:C], inputs["ca_kv_b"][C:2 * C]
    cklb, cvlb = inputs["ca_kv_lb"][0:C], inputs["ca_kv_lb"][C:2 * C]
    g["wck"] = bf(np.asarray(ckw).T)
    g["wcv"] = bf(np.asarray(cvw).T)
    g["b_ckk"] = baug(np.asarray(cklb).T * SCALE, ckb)
    g["b_ckv"] = baug(np.asarray(cvlb).T * SCALE, cvb)

    g["wcp"] = bf(np.asarray(inputs["ca_proj_w"]).T)
    g["b_cp"] = baug(np.asarray(inputs["ca_proj_lb"]).T * SCALE, inputs["ca_proj_b"])

    g["wfc"] = bf(np.asarray(inputs["fc_w"]).T)
    g["wpr"] = bf(np.asarray(inputs["pr_w"]).T)

    # apack: all lora A^T chunk-columns, partition-major
    apack = np.zeros((128, len(A_TAGS) * NT * R), np.float32)
    a_srcs = {
        "a_sa": inputs["sa_qkv_a"], "a_sp": inputs["sa_proj_a"],
        "a_cq": inputs["ca_q_a"], "a_ck": inputs["ca_kv_a"], "a_cp": inputs["ca_proj_a"],
    }
    for ti, tag in enumerate(A_TAGS):
        at = np.asarray(a_srcs[tag]).T  # [C, R]
        for k in range(NT):
            apack[:, (ti * NT + k) * R:(ti * NT + k + 1) * R] = at[k * 128:(k + 1) * 128]
    g["apack"] = bf(apack)

    # bcols: per-column LN affine + pr/fc biases, partition-major
    bcols = np.zeros((128, 72), np.float32)

    def fill(base, vec, n=NT):
        v = np.asarray(vec, np.float32).reshape(n, 128).T  # [128, n]
        bcols[:, base:base + n] = v

    fill(BC_G1, inputs["ln1_g"])
    fill(BC_B1, inputs["ln1_b"])
    fill(BC_G2, inputs["ln2_g"])
    fill(BC_B2, inputs["ln2_b"])
    fill(BC_BPR, inputs["pr_b"])
    fill(BC_BFC, inputs["fc_b"], 32)
    g["bcols"] = bcols

    sel = np.zeros((8, NT * 128), np.float32)
    for mi in range(NT):
        rA = 2 * (mi % 4)
        sel[rA, mi * 128:mi * 128 + 64] = 1.0
        sel[rA + 1, mi * 128 + 64:(mi + 1) * 128] = 1.0
    g["sel"] = sel
    return g


def _make_in_maps(inputs):
    inputs = {k: np.asarray(v) for k, v in inputs.items()}
    x, feat = inputs["x"], inputs["feature"]
    B = x.shape[0]
    shared = _prep_shared(inputs)

    bands = []
    for p in range(2):
        jj = np.arange(128).reshape(128, 1)
        ii = np.arange(64).reshape(1, 64)
        b01 = (jj <= 2 * ii + p).astype(np.float32)
        bands.append(np.concatenate([b01, b01], axis=1).astype(BF))  # [128,128]

    in_maps = []
    xTs = [np.ascontiguousarray(np.asarray(x[b]).T, dtype=np.float32) for b in range(B)]
    fTs = [np.ascontiguousarray(np.asarray(feat[b]).T, dtype=np.float32) for b in range(B)]
    for core in range(NCORES):
        b, p = core // 2, core % 2
        m = dict(shared)
        m["xT"] = xTs[b]
        m["xqT"] = np.ascontiguousarray(xTs[b][:, p::2])
        m["fT"] = fTs[b]
        m["band2"] = bands[p]
        in_maps.append(m)
    return in_maps, B


def kernel(**inputs):
    from concourse.bass_utils import run_bass_kernel_spmd

    nc = _get_program()
    in_maps, B = _make_in_maps(inputs)
    res = run_bass_kernel_spmd(nc, in_maps, core_ids=list(range(NCORES)))
    out = np.zeros((B, T, C), np.float32)
    for core in range(NCORES):
        b, p = core // 2, core % 2
        out[b, p::2, :] = np.asarray(res.results[core]["outT"], dtype=np.float32).T
    return out
